# revision 18
# baseline (speedup 1.0000x reference)
"""Multi-head attention (B=4, S=2048, D=1024, H=16, causal) on 8 trn2 cores.

Sharding: core c = (batch b = c//2, head-group g = c%2). Each core computes
the QKV projections for its 8 heads on its batch, causal flash-style
attention (unnormalized exp + deferred 1/rowsum), and a partial output
projection over its 512 head-dims. Host sums the two partials per batch and
adds the bias.

Matmul operands are fp16 (same 10-bit mantissa as TF32; all values here are
far below fp16 max) with fp32 PSUM accumulation.

Attention is processed in HEAD PAIRS (2h, 2h+1): the pair's kT/qT slices
live at partition offsets 0 and 64 of the same SBUF tile, so the two
heads' score matmuls (contraction = DK = 64) auto-derive PE tile_positions
(0,0) and (64,0) and execute CONCURRENTLY on disjoint row-halves of the
128x128 PE array - 2x throughput on the scores vs serial heads.

Softmax max-subtraction is skipped (scores ~ N(0,1); exp cannot overflow;
softmax is shift-invariant). Normalization is deferred and done per head
pair: attention accumulates unnormalized y plus row-sums l (ones column
appended to V -> PSUM partition 64), the pair's two l rows are gathered to
a [2, 512] tile, inverted with a 1-pass approx reciprocal (18-bit, far
below the fp16 data precision), broadcast 2->128 partitions with a tiny
K=2 block-indicator matmul, and multiplied into the pair's y tile reading
the broadcast directly from PSUM. Causal masking of diagonal-straddling
attn tiles runs as affine_select on the otherwise-idle GpSimd engine;
fully-masked tiles are never computed.

A short chain of throwaway matmuls on a zeroed SBUF tile runs at t=0 so
the PE's HAM clock-gate warms up (and stays busy) during the initial
weight/activation DMA wait instead of starting the first projection round
at the cold 1.2 GHz clock.
"""

import sys

if "/opt/trn_rl_repo" not in sys.path:
    sys.path.insert(0, "/opt/trn_rl_repo")

from contextlib import ExitStack

import numpy as np

import concourse.bacc as bacc
import concourse.mybir as mybir
import concourse.tile as tile
from concourse.bass_utils import run_bass_kernel_spmd

B, S, D = 4, 2048, 1024
H, DK = 16, 64
G = 2  # head groups (tensor parallel)
HPG = H // G  # 8 heads per core
HD = HPG * DK  # 512 head dims per core
NC = 8
P = 128
NT = S // P  # 16 token chunks of 128
NJ = S // 512  # 4 query blocks of 512
KC = D // P  # 8 d_model chunks
MC = HD // P  # 4 head-dim chunks (= head pairs)

F32 = mybir.dt.float32
DT = mybir.dt.float16
NPDT = np.float16
EXP = mybir.ActivationFunctionType.Exp

N_WARM = 40  # PE warmup matmuls issued during the initial DMA wait

_CACHE = {}


def _emat2():
    # block-indicator for broadcasting a head pair's [2, 512] 1/l rows to
    # [128, 512]: row 0 -> partitions 0-63 (head A), row 1 -> 64-127 (B)
    e = np.zeros((2, P), dtype=NPDT)
    e[0, 0:64] = 1.0
    e[1, 64:128] = 1.0
    return e


def _build():
    nc = bacc.Bacc("TRN2", target_bir_lowering=False, debug=False)

    xqT = nc.dram_tensor("xqT", [D, S], DT, kind="ExternalInput")
    xkT = nc.dram_tensor("xkT", [D, S], DT, kind="ExternalInput")
    xvT = nc.dram_tensor("xvT", [D, S], DT, kind="ExternalInput")
    wqT = nc.dram_tensor("wqT", [D, HD], DT, kind="ExternalInput")
    wkT = nc.dram_tensor("wkT", [D, HD], DT, kind="ExternalInput")
    wvT = nc.dram_tensor("wvT", [D, HD], DT, kind="ExternalInput")
    wpg = nc.dram_tensor("wpg", [HD, D], DT, kind="ExternalInput")
    ein = nc.dram_tensor("ein", [2, P], DT, kind="ExternalInput")
    out = nc.dram_tensor("out", [S, D], F32, kind="ExternalOutput")

    with tile.TileContext(nc) as tc, ExitStack() as ctx:
        persist = ctx.enter_context(tc.tile_pool(name="persist", bufs=1))

        qT = [persist.tile([P, S], DT, name=f"qT{m}", tag=f"qT{m}") for m in range(MC)]
        kT = [persist.tile([P, S], DT, name=f"kT{m}", tag=f"kT{m}") for m in range(MC)]
        vext = [
            persist.tile([P, HPG, 66], DT, name=f"vext{t}", tag=f"vext{t}")
            for t in range(NT)
        ]
        emat = persist.tile([2, P], DT, name="emat", tag="emat")
        wp_sb = persist.tile([P, MC, D], DT, name="wp_sb", tag="wp_sb")
        wq_sb = persist.tile([P, KC, HD], DT, name="wq_sb", tag="wq_sb")
        wk_sb = persist.tile([P, KC, HD], DT, name="wk_sb", tag="wk_sb")
        wv_sb = persist.tile([P, KC, HD], DT, name="wv_sb", tag="wv_sb")
        wsrc = persist.tile([P, 512], DT, name="wsrc", tag="wsrc")

        nc.sync.dma_start(
            out=wq_sb[:, 0:2, :],
            in_=wqT.ap()[0 : 2 * P, :].rearrange("(c p) n -> p c n", p=P),
        )
        nc.scalar.dma_start(
            out=wq_sb[:, 2:KC, :],
            in_=wqT.ap()[2 * P :, :].rearrange("(c p) n -> p c n", p=P),
        )

        with tc.tile_pool(name="init", bufs=1) as initpool:
            onecol = initpool.tile([P, HPG], F32, name="onecol", tag="onecol")
            nc.vector.memset(onecol[:], 1.0)
            nc.vector.memset(wsrc[:], 0.0)
            for t in range(NT):
                nc.vector.tensor_copy(
                    vext[t][:, :, 64:65],
                    onecol[:].rearrange("p (h o) -> p h o", o=1),
                )

        with (
            tc.tile_pool(name="psA", bufs=2, space="PSUM") as psA,
            tc.tile_pool(name="ps_s", bufs=2, space="PSUM") as ps_s,
            tc.tile_pool(name="ps_acc", bufs=2, space="PSUM") as ps_acc,
            tc.tile_pool(name="xpool", bufs=3) as xpool,
            tc.tile_pool(name="attn", bufs=8) as attn_pool,
            tc.tile_pool(name="ypool", bufs=2) as ypool,
            tc.tile_pool(name="rpool", bufs=2) as rpool,
            tc.tile_pool(name="opool", bufs=3) as opool,
        ):
            # PE warmup: ~10us of matmuls on zeroed data so the HAM clock
            # gate reaches 8/8 while the weight/x DMAs are in flight
            for w in range(N_WARM):
                pw = psA.tile([P, 512], F32, name="pw", tag="psA")
                nc.tensor.matmul(
                    pw[:], wsrc[:, 0:128], wsrc[:], start=True, stop=True
                )

            xcur = {}

            def proj_qk_chunk(rnd, m):
                # q[m] + k[m] for token block rnd: emitted interleaved with
                # the previous round's attention pairs so the NEXT round's
                # first score inputs (m=0) are ready long before its exps
                # are due, keeping the scalar engine's exp stream dense
                # across round boundaries
                if m == 0:
                    xts = {}
                    for pref, xin in (("q", xqT), ("k", xkT)):
                        xt = xpool.tile(
                            [P, KC, 512], DT, name=f"x{pref}", tag=f"x{pref}"
                        )
                        nc.sync.dma_start(
                            out=xt[:, 0:2, :],
                            in_=xin.ap()[
                                0 : 2 * P, rnd * 512 : (rnd + 1) * 512
                            ].rearrange("(c p) n -> p c n", p=P),
                        )
                        nc.sync.dma_start(
                            out=xt[:, 2:KC, :],
                            in_=xin.ap()[
                                2 * P :, rnd * 512 : (rnd + 1) * 512
                            ].rearrange("(c p) n -> p c n", p=P),
                        )
                        xts[pref] = xt
                    xcur[rnd] = xts
                xts = xcur[rnd]
                for pref, w_sb, dst in (("q", wq_sb, qT), ("k", wk_sb, kT)):
                    xt = xts[pref]
                    pt = psA.tile([P, 512], F32, name="psA", tag="psA")
                    for kc in range(KC):
                        nc.tensor.matmul(
                            pt[:],
                            w_sb[:, kc, m * P : (m + 1) * P],
                            xt[:, kc, :],
                            start=(kc == 0),
                            stop=(kc == KC - 1),
                        )
                    nc.vector.tensor_copy(
                        dst[m][:, rnd * 512 : (rnd + 1) * 512], pt[:]
                    )

            def proj_v(rnd):
                # v for key-token chunks of block rnd (shared x tile)
                xtv = xpool.tile([P, KC, 512], DT, name="xtv", tag="xtv")
                nc.sync.dma_start(
                    out=xtv[:],
                    in_=xvT.ap()[:, rnd * 512 : (rnd + 1) * 512].rearrange(
                        "(c p) n -> p c n", p=P
                    ),
                )
                for t in range(4 * rnd, 4 * rnd + 4):
                    tt = t % 4
                    pv = psA.tile([P, 512], F32, name="psV", tag="psA")
                    for kc in range(KC):
                        nc.tensor.matmul(
                            pv[:],
                            xtv[:, kc, tt * P : (tt + 1) * P],
                            wv_sb[:, kc, :],
                            start=(kc == 0),
                            stop=(kc == KC - 1),
                        )
                    nc.vector.tensor_copy(
                        vext[t][:, :, 0:64],
                        pv[:].rearrange("p (h d) -> p h d", h=HPG),
                    )

            def oproj(j, ytiles):
                # partial out-projection for query block j
                for nd in range(2):
                    for mt in range(4):
                        po = psA.tile([P, 512], F32, name="po", tag="psA")
                        for c in range(MC):
                            nc.tensor.matmul(
                                po[:],
                                ytiles[c][:, mt * P : (mt + 1) * P],
                                wp_sb[:, c, nd * 512 : (nd + 1) * 512],
                                start=(c == 0),
                                stop=(c == MC - 1),
                            )
                        ot = opool.tile([P, 512], F32, name="ot", tag="ot")
                        nc.vector.tensor_copy(ot[:], po[:])
                        nc.sync.dma_start(
                            out=out.ap()[
                                j * 512 + mt * P : j * 512 + (mt + 1) * P,
                                nd * 512 : (nd + 1) * 512,
                            ],
                            in_=ot[:],
                        )

            nc.scalar.dma_start(
                out=wk_sb[:], in_=wkT.ap().rearrange("(c p) n -> p c n", p=P)
            )
            nc.gpsimd.dma_start(
                out=wv_sb[:], in_=wvT.ap().rearrange("(c p) n -> p c n", p=P)
            )
            nc.gpsimd.dma_start(out=emat[:], in_=ein.ap())
            proj_qk_chunk(0, 0)
            proj_v(0)

            prev_ytiles = None
            for rnd in range(NJ):
                # ---- attention for query block j = rnd, by head pair ----
                j = rnd
                ilast = 4 * j + 3
                ytiles = [
                    ypool.tile([P, 512], DT, name=f"y{c}", tag=f"y{c}")
                    for c in range(MC)
                ]
                for h2 in range(MC):
                    hA, hB = 2 * h2, 2 * h2 + 1
                    pyA = ps_acc.tile([65, 512], F32, name="pyA", tag="acc")
                    pyB = ps_acc.tile([65, 512], F32, name="pyB", tag="acc")
                    for i0 in range(0, ilast + 1, 2):
                        # columns < trim are fully causally masked for tile
                        # i; skip computing them (stale PSUM there is later
                        # ignored: the AV matmul reads from trim onward)
                        trims = [max(0, 128 * (i0 + z) - 512 * j) for z in (0, 1)]
                        # score tiles grouped by z: [head A | head B].  The
                        # pair's two score matmuls then share one WAR event
                        # (the z-exp of the previous slot user), become
                        # ready together, issue adjacently, and run
                        # CONCURRENTLY on disjoint PE row-halves
                        psc = [
                            ps_s.tile([P, 1024], F32, name=nm, tag="pssc")
                            for nm in ("pscz0", "pscz1")
                        ]
                        ats = [
                            attn_pool.tile([P, 1024], DT, name=nm, tag="at")
                            for nm in ("atz0", "atz1")
                        ]
                        for z in (0, 1):
                            i = i0 + z
                            tr = trims[z]
                            for x, poff in ((0, 0), (1, 64)):
                                nc.tensor.matmul(
                                    psc[z][:, x * 512 + tr : (x + 1) * 512],
                                    kT[h2][
                                        poff : poff + 64, i * P : (i + 1) * P
                                    ],
                                    qT[h2][
                                        poff : poff + 64,
                                        j * 512 + tr : (j + 1) * 512,
                                    ],
                                    start=True,
                                    stop=True,
                                )
                        for z in (0, 1):
                            nc.scalar.activation(
                                out=ats[z][:, trims[z] : 1024],
                                in_=psc[z][:, trims[z] : 1024],
                                func=EXP,
                                scale=0.125,
                            )
                        for z in (0, 1):
                            i = i0 + z
                            d = 128 * i - 512 * j
                            tr = trims[z]
                            if d >= 0:  # diagonal-straddling: causal mask
                                for x in (0, 1):
                                    nc.gpsimd.affine_select(
                                        out=ats[z][
                                            :, x * 512 + tr : (x + 1) * 512
                                        ],
                                        in_=ats[z][
                                            :, x * 512 + tr : (x + 1) * 512
                                        ],
                                        compare_op=mybir.AluOpType.is_ge,
                                        fill=0.0,
                                        base=tr - d,
                                        pattern=[[1, 512 - tr]],
                                        channel_multiplier=-1,
                                    )  # keep where sq >= sk
                        for x, h, py in ((0, hA, pyA), (1, hB, pyB)):
                            for z in (0, 1):
                                i = i0 + z
                                tr = trims[z]
                                nc.tensor.matmul(
                                    py[:, tr:512],
                                    vext[i][:, h, 0:65],
                                    ats[z][:, x * 512 + tr : (x + 1) * 512],
                                    start=(i == 0),
                                    stop=(i == ilast),
                                )

                    # ---- normalize this pair (deferred 1/l) ----
                    lr2 = rpool.tile([2, 512], F32, name="lr2", tag="lr2")
                    nc.vector.tensor_copy(lr2[0:1, :], pyA[64:65, :])
                    ltB = rpool.tile([1, 512], F32, name="ltB", tag="ltB")
                    nc.vector.tensor_copy(ltB[:], pyB[64:65, :])
                    nc.gpsimd.dma_start(out=lr2[1:2, :], in_=ltB[:])
                    rinv2 = rpool.tile([2, 512], F32, name="rinv2", tag="rinv2")
                    nc.vector.reciprocal_approx_fast(
                        out=rinv2[0:2, :], in_=lr2[0:2, :]
                    )
                    rr16 = rpool.tile([2, 512], DT, name="rr16", tag="rr16")
                    nc.vector.tensor_copy(rr16[0:2, :], rinv2[0:2, :])
                    pr = psA.tile([P, 512], F32, name="pr", tag="psA")
                    nc.tensor.matmul(
                        pr[:], emat[0:2, :], rr16[0:2, :], start=True, stop=True
                    )
                    nc.vector.tensor_copy(ytiles[h2][0:64, :], pyA[0:64, :])
                    nc.vector.tensor_copy(ytiles[h2][64:128, :], pyB[0:64, :])
                    nc.vector.tensor_mul(ytiles[h2][:], ytiles[h2][:], pr[:])
                    # PE filler emitted AFTER each pair: pops in the idle
                    # slots of later pairs' exp waits, and never stands
                    # between a round/pair boundary and the next scores
                    if rnd == 0:
                        # round 0 runs its own chunks one slot behind so
                        # only qk[0] stands before the first exps; each
                        # chunk still lands a full pair ahead of its reader
                        if h2 == 0:
                            proj_qk_chunk(0, 1)
                        elif h2 == 1:
                            proj_qk_chunk(0, 2)
                            nc.gpsimd.dma_start(
                                out=wp_sb[:],
                                in_=wpg.ap().rearrange("(c p) n -> p c n", p=P),
                            )
                        elif h2 == 2:
                            proj_qk_chunk(0, 3)
                            proj_qk_chunk(1, 0)
                            proj_v(1)
                        else:
                            proj_qk_chunk(1, 1)
                    elif h2 == 0:
                        proj_qk_chunk(rnd, 2)
                        oproj(rnd - 1, prev_ytiles)
                    elif h2 == 1:
                        proj_qk_chunk(rnd, 3)
                        if rnd + 1 < NJ:
                            proj_v(rnd + 1)
                    elif h2 == 2 and rnd + 1 < NJ:
                        proj_qk_chunk(rnd + 1, 0)
                    elif h2 == 3 and rnd + 1 < NJ:
                        proj_qk_chunk(rnd + 1, 1)
                prev_ytiles = ytiles
            oproj(NJ - 1, prev_ytiles)

    nc.compile()
    return nc



def kernel(query_data, key_data, value_data, Wq, Wk, Wv, Wp, bp):
    query_data = np.asarray(query_data, dtype=np.float32)
    key_data = np.asarray(key_data, dtype=np.float32)
    value_data = np.asarray(value_data, dtype=np.float32)
    Wq = np.asarray(Wq, dtype=np.float32)
    Wk = np.asarray(Wk, dtype=np.float32)
    Wv = np.asarray(Wv, dtype=np.float32)
    Wp = np.asarray(Wp, dtype=np.float32)
    bp = np.asarray(bp, dtype=np.float32)

    if "nc" not in _CACHE:
        _CACHE["nc"] = _build()
    nc = _CACHE["nc"]

    in_maps = []
    for c in range(NC):
        b, g = divmod(c, G)
        sl = slice(g * HD, (g + 1) * HD)
        in_maps.append(
            {
                "xqT": np.ascontiguousarray(query_data[b].T).astype(NPDT),
                "xkT": np.ascontiguousarray(key_data[b].T).astype(NPDT),
                "xvT": np.ascontiguousarray(value_data[b].T).astype(NPDT),
                "wqT": np.ascontiguousarray(Wq[sl, :].T).astype(NPDT),
                "wkT": np.ascontiguousarray(Wk[sl, :].T).astype(NPDT),
                "wvT": np.ascontiguousarray(Wv[sl, :].T).astype(NPDT),
                "wpg": np.ascontiguousarray(Wp[:, sl].T).astype(NPDT),
                "ein": _emat2(),
            }
        )

    res = run_bass_kernel_spmd(nc, in_maps, core_ids=list(range(NC)))
    _CACHE["last_results"] = res

    out = np.zeros((B, S, D), dtype=np.float32)
    for c in range(NC):
        b = c // G
        out[b] += res.results[c]["out"]
    out += bp
    return out
